# revision 19
# baseline (speedup 1.0000x reference)
"""Causal multi-head self-attention on 8 TRN2 NeuronCores.

Problem: x (2, 2048, 1024) f32; w_q/w_k/w_v/w_o (1024, 1024) f32;
out = CausalMHA(x) (torch nn.Linear convention, 16 heads, d_k = 64).

Sharding (tensor-parallel over heads x data-parallel over batch):
core c -> batch bc = c//4, head group hg = c%4 (4 heads = 256 features).
Each core computes Q/K/V projections for its slice, causal attention for
its 4 heads, and a partial output projection against its w_o column
slice. The host sums the 4 partials per batch (the tensor-parallel
"all-reduce" of the w_o matmul, done host-side during unshard).

Device kernel (per core, one NEFF, all matmuls bf16 w/ f32 PSUM accum).

The PE p-state is the dominant performance lever: a dense matmul stream
runs at 2.4 GHz (0.42 ns/col) but ANY pipeline bubble drops the clock
to 1.2 GHz for the next ~3 us of ramp.  The schedule is therefore built
around gaplessness:
- scores computed K-major (ST layout [kv, q]); head pairs in PE row
  groups (0,0)/(64,0) run concurrently (K=64 each).
- exp is COLUMN-SPLIT between ScalarE (exact exp, left slice + the
  diagonal strip of band tiles) and DVE (Schraudolph bits-trick: one
  tensor_scalar computing round(score*23.0831 + 16256) into a uint16
  view of the bf16 P tile -- exp(score/8) to within +-3%, zero-mean
  differential after the softmax normalize).  Halves the exp latency
  per tile and removes ScalarE as co-bottleneck.
- scores(kt+1) are emitted BEFORE av(kt) so the AV matmuls never head
  the PE queue while their exp is still in flight.
- softmax denominators via the [64 ones | 64 V] AV stationary; the
  normalize (reciprocal_approx_fast + tensor_mul) runs progressively
  (after band tiles jj=1 and jj=3) on every chunk so the av PSUM banks
  free shortly after their last AV and the WAR edge to the next head
  pair never stalls the PE.
- causal masking: band tiles narrow the matmul/exp to [s:TC]; the
  diagonal 128-col strip is exp'd exactly on ScalarE then triangular-
  zeroed by GPSIMD affine_select.
- all PSUM->SBUF copies (Q/K/V proj results, outproj casts) ride
  ScalarE (activation Copy, same act table as Exp -> no table reloads);
  DVE does only exp + normalize.
- proj / outproj groups are statically interleaved as filler between
  attention k-tile groups so TensorE never idles (keeping the HAM
  clock at 8/8).
- x is DMAd with one descriptor per 512-token chunk (first chunk in two
  halves), ordered wq, x0, wk, x1.. so the first proj starts early.
- output returned bf16 (cast to f32 host-side); error budget dominated
  by the bf16 input quantization (~4e-3 max-relative overall).
"""

import numpy as np
import ml_dtypes

import concourse.bass as bass
import concourse.tile as tile
from concourse import bacc, mybir
from concourse.bass import ts

P = 128
D = 1024          # d_model
T = 2048          # seq len
B = 2
NH = 4            # heads per core
DK = 64
F = NH * DK       # 256 local features
TC = 512          # token chunk (matmul N)
NCHUNK = T // TC  # 4
NTT = T // P      # 16 token tiles
KA = D // P       # 8 dmodel chunks
SCALE = 1.0 / np.sqrt(DK)

BF16 = mybir.dt.bfloat16
F32 = mybir.dt.float32
U16 = mybir.dt.uint16
EXP = mybir.ActivationFunctionType.Exp
ALU = mybir.AluOpType

# Schraudolph constants: bf16 bits of exp(score * SCALE)
#   bits16 = round(score * SCALE * 128 * log2(e) + 16256)
# DVE f32->u16 conversion is RNE + saturating (measured), and scores
# stay within +-50 so bits16 is always in [15100, 17400]: no overflow,
# no zero-flush, p > 0 for any finite score (no 0/0 softmax rows).
SCHRAUD_A = float(SCALE * 128.0 * np.log2(np.e))  # 23.08312
SCHRAUD_B = 16256.0
# column split of each exp tile: ScalarE takes [c0, c0+SPLIT_S), DVE the
# rest (diagonal strips always ScalarE, on top of this share)
SPLIT_S = 224


def build_nc(sim_safe=None):
    import os
    if sim_safe is None:
        sim_safe = bool(int(os.environ.get("KSIM_SAFE", "0")))
    nc = bacc.Bacc(None, target_bir_lowering=False)
    with tile.TileContext(nc) as tc:
        with tc.tile_pool(name="dram", bufs=1, space="DRAM") as dram:
            xT = dram.tile((P, NCHUNK, KA, TC), BF16, kind="ExternalInput", name="xT", uniquify=False)
            wqT = dram.tile((P, KA, F), BF16, kind="ExternalInput", name="wqT", uniquify=False)
            wkT = dram.tile((P, KA, F), BF16, kind="ExternalInput", name="wkT", uniquify=False)
            wvT = dram.tile((P, KA, F), BF16, kind="ExternalInput", name="wvT", uniquify=False)
            woT = dram.tile((P, F // P, D), BF16, kind="ExternalInput", name="woT", uniquify=False)
            out = dram.tile((P, NTT, D), BF16, kind="ExternalOutput", name="out", uniquify=False)

            with tc.tile_pool(name="big", bufs=1) as big:
                xT_sb = big.tile([P, NCHUNK, KA, TC], BF16)
                wqT_sb = big.tile([P, KA, F], BF16)
                wkT_sb = big.tile([P, KA, F], BF16)
                wvT_sb = big.tile([P, KA, F], BF16)
                woT_sb = big.tile([P, F // P, D], BF16)
                QT_sb = big.tile([P, 2, T], BF16)   # head pair-major
                KT_sb = big.tile([P, 2, T], BF16)
                V_sb = big.tile([P, NTT, NH, 128], BF16)  # [ones(64) | V(64)]
                # per-chunk Y tiles: separate tiles so outproj bodies for
                # chunk c never pick up a false dependency on a later
                # chunk's normalize writes.
                YT_c = [big.tile([P, 2, TC], BF16, name=f"YT{c}") for c in range(NCHUNK)]

                # All loads on the sync ring, in consumption order, so the
                # DMA engine drains them with first-proj data first (x is
                # chunk-contiguous in DRAM -> 8KB descriptor lines).
                nc.sync.dma_start(out=wqT_sb[:, 0:4], in_=wqT[:, 0:4])
                nc.sync.dma_start(out=xT_sb[:, 0, 0:4], in_=xT[:, 0, 0:4])
                nc.sync.dma_start(out=wqT_sb[:, 4:8], in_=wqT[:, 4:8])
                nc.sync.dma_start(out=xT_sb[:, 0, 4:8], in_=xT[:, 0, 4:8])
                nc.sync.dma_start(out=wkT_sb[:], in_=wkT[:])
                nc.sync.dma_start(out=wvT_sb[:], in_=wvT[:])
                nc.sync.dma_start(out=xT_sb[:, 1], in_=xT[:, 1])
                nc.sync.dma_start(out=woT_sb[:], in_=woT[:])
                nc.sync.dma_start(out=xT_sb[:, 2], in_=xT[:, 2])
                nc.sync.dma_start(out=xT_sb[:, 3], in_=xT[:, 3])
                nc.gpsimd.memset(V_sb[:, :, :, 0:64], 1.0)

                with (tc.tile_pool(name="flex", bufs=2, space="PSUM") as flexp,
                      tc.tile_pool(name="st", bufs=2, space="PSUM") as stp,
                      tc.tile_pool(name="av", bufs=2, space="PSUM") as avp,
                      tc.tile_pool(name="pt", bufs=4) as ptp,
                      tc.tile_pool(name="sm", bufs=4) as smp,
                      tc.tile_pool(name="warm", bufs=1) as warmp,
                      tc.tile_pool(name="ob", bufs=4) as obp):

                    if True:  # pre-warm ACT exp table during DMA phase
                        wt = warmp.tile([1, 8], F32)
                        nc.vector.memset(wt[:], 0.0)
                        nc.scalar.activation(wt[:], wt[:], EXP, scale=1.0)

                    if True:
                        # warm-up matmuls on zeroed SBUF while the x DMAs
                        # land: ~3.4us of sustained PE activity flips the
                        # HAM clock gate to 8/8 before the first real
                        # matmul, which otherwise runs its first ~4us at
                        # 1.2 GHz.
                        wsrc = warmp.tile([P, TC], BF16)
                        nc.vector.memset(wsrc[:], 0.0)
                        wps = stp.tile([P, 2, TC], F32, name="st_ps")
                        for _ in range(28):
                            nc.tensor.matmul(
                                wps[:, 0, :], lhsT=wsrc[:, 0:P], rhs=wsrc[:],
                                start=True, stop=True)
                        # read the warm tile: a reader-less PSUM tile gives
                        # the pool no WAR edge, so the scheduler could
                        # interleave the first real score matmuls with the
                        # warm-up group on the same banks (observed as a
                        # nondeterministic correctness race).
                        nc.vector.tensor_copy(wt[:], wps[0:1, 0, 0:8])

                    def qk_group(n, pr, which):
                        w_sb = wqT_sb if which == "q" else wkT_sb
                        dst = QT_sb if which == "q" else KT_sb
                        def emit():
                            ps = flexp.tile([P, TC], F32, name="flex")
                            for a in range(KA):
                                nc.tensor.matmul(
                                    ps[:], lhsT=w_sb[:, a, ts(pr, P)],
                                    rhs=xT_sb[:, n, a, :],
                                    start=(a == 0), stop=(a == KA - 1))
                            nc.scalar.copy(dst[:, pr, ts(n, TC)], ps[:])
                        return emit

                    def v_group(tt):
                        def emit():
                            ps_v = flexp.tile([P, TC], F32, name="flex")
                            for a in range(KA):
                                nc.tensor.matmul(
                                    ps_v[:, 0:F], lhsT=xT_sb[:, tt // 4, a, ts(tt % 4, P)],
                                    rhs=wvT_sb[:, a, :],
                                    start=(a == 0), stop=(a == KA - 1))
                            nc.scalar.copy(
                                V_sb[:, tt, :, 64:128],
                                ps_v[:, 0:F].rearrange("p (h d) -> p h d", h=NH))
                        return emit

                    def proj_groups(n):
                        gs = [qk_group(n, pr, w) for pr in range(2) for w in ("q", "k")]
                        gs += [v_group(tt) for tt in range(4 * n, 4 * n + 4)]
                        return gs

                    def proj(n):
                        for g in proj_groups(n):
                            g()

                    fill_proj = []   # next chunk's proj groups: must drain in-chunk
                    fill_op = []     # outproj bodies: carry across chunks
                    _ob_cache = {}

                    def emit_filler(k=1):
                        for _ in range(k):
                            if fill_proj:
                                fill_proj.pop(0)()
                            elif fill_op:
                                fill_op.pop(0)()

                    def drain_proj():
                        while fill_proj:
                            fill_proj.pop(0)()

                    def emit_exp(st_ps, pt_sb, s, band, split=SPLIT_S):
                        """Column-split exp of st_ps[:, :, s:TC] into pt_sb.

                        ScalarE: exact exp on the diagonal strip (if band)
                        merged with its share of flex columns; DVE:
                        Schraudolph bits-trick on the rest.  Mixed tiles are
                        consistent: both compute exp(score*SCALE), no bias.
                        """
                        if band:
                            cs = 128 + max(0, min(TC - s - 128, split - 128))
                        else:
                            cs = min(TC - s, split)
                        nc.scalar.activation(
                            pt_sb[:, :, s:s + cs], st_ps[:, :, s:s + cs],
                            EXP, scale=float(SCALE))
                        if s + cs < TC:
                            nc.vector.tensor_scalar(
                                pt_sb[:, :, s + cs:TC].bitcast(U16),
                                st_ps[:, :, s + cs:TC],
                                SCHRAUD_A, SCHRAUD_B, ALU.mult, ALU.add)
                        if band:
                            # one call masks both heads: iota is
                            # col - partition, independent of j.
                            nc.gpsimd.affine_select(
                                out=pt_sb[:, :, s:s + 128],
                                in_=pt_sb[:, :, s:s + 128],
                                compare_op=mybir.AluOpType.is_ge,
                                fill=0.0, base=0,
                                pattern=[[0, 2], [1, 128]],
                                channel_multiplier=-1)

                    def normalize_blk(av_ps, r, hp, n, c0, c1, mul_eng=None):
                        # av rows 0:64 = 64 broadcast copies of the row
                        # sums (ones block of the AV stationary); rows
                        # 64:128 = unnormalized Y.  Columns [c0:c1) are
                        # final once the band tile covering them has
                        # accumulated, so this reciprocal+scale overlaps
                        # the remaining k-tiles.
                        rec = smp.tile([64, c1 - c0], F32, name=f"rec{c1 - c0}")
                        nc.vector.reciprocal_approx_fast(
                            out=rec[:], in_=av_ps[0:64, c0:c1])
                        (mul_eng or nc.vector).tensor_mul(
                            YT_c[n][r:r + 64, hp, c0:c1],
                            av_ps[64:128, c0:c1],
                            rec[:])

                    _pend_norm = []

                    def flush_norms():
                        for fn in _pend_norm:
                            fn()
                        _pend_norm.clear()

                    # how many outproj filler bodies each chunk may consume
                    OP_BUDGET = [0, 0, 2, 99]

                    def attention(n):
                        # head pairs processed together: the two K=64 score
                        # matmuls go to distinct PE row groups (0,0)/(64,0)
                        # and run concurrently in the systolic array.
                        last_kt = 4 * n + 3
                        nkt = 4 * n + 4
                        slots = 2 * (nkt + 1)
                        budget = len(fill_proj) + min(len(fill_op), OP_BUDGET[n])
                        pace_state = [0, 0]  # slot index, emitted

                        def pace(force_min=0):
                            pace_state[0] += 1
                            want = pace_state[0] * budget // slots
                            k = max(force_min, want - pace_state[1])
                            pace_state[1] += k
                            emit_filler(k)

                        for hp in range(2):
                            av_a = avp.tile([P, TC], F32, name="av_ps")
                            av_b = avp.tile([P, TC], F32, name="av_ps")

                            def scores(kt, hp=hp):
                                jj = kt - 4 * n
                                band = (jj >= 0)
                                s = 128 * jj if band else 0
                                st_ps = stp.tile([P, 2, TC], F32, name="st_ps")
                                pt_sb = ptp.tile([P, 2, TC], BF16, name="pt_sb")
                                for j, r in ((0, 0), (1, 64)):
                                    nc.tensor.matmul(
                                        st_ps[:, j, s:TC],
                                        lhsT=KT_sb[r:r + 64, hp, ts(kt, P)],
                                        rhs=QT_sb[r:r + 64, hp, n * TC + s:(n + 1) * TC],
                                        start=True, stop=True)
                                emit_exp(st_ps, pt_sb, s, band,
                                         split=256 if n == NCHUNK - 1 else SPLIT_S)
                                return pt_sb, s, band, jj

                            def av(kt, pt_sb, s, hp=hp, av_a=av_a, av_b=av_b):
                                for j, avt in ((0, av_a), (1, av_b)):
                                    nc.tensor.matmul(
                                        avt[:, s:TC],
                                        lhsT=V_sb[:, kt, 2 * hp + j, :],
                                        rhs=pt_sb[:, j, s:TC],
                                        start=(kt == 0), stop=(kt == last_kt))

                            KEG = int(__import__('os').environ.get('KEG', '3'))
                            endgame = (hp == 1 and n == NCHUNK - 1) and KEG > 0
                            # software pipeline: scores/exp run one k-tile
                            # ahead of the AV matmuls, and the PREVIOUS head
                            # pair's final normalize is flushed only after
                            # this pair's first exp is in the engine queues,
                            # so the seam never stalls the PE on a
                            # serialized DVE chain.
                            pend = scores(0)
                            flush_norms()
                            pace(force_min=2 if n >= NCHUNK - 1 else 1)
                            for kt in range(nkt):
                                if kt + 1 < nkt:
                                    nxt = scores(kt + 1)
                                else:
                                    nxt = None
                                    if endgame and not sim_safe:
                                        # outproj of tokens [128:256) covers
                                        # the last AV's exp window
                                        _op_body(4 * n + 1)
                                pace()
                                pt_sb, s, band, jj = pend
                                av(kt, pt_sb, s)
                                if band and not sim_safe:
                                    if jj == 2:
                                        normalize_blk(av_a, 0, hp, n, 0, 256)
                                        normalize_blk(av_b, 64, hp, n, 0, 256)
                                        if endgame:
                                            _op_body(4 * n)
                                    elif jj == 3:
                                        normalize_blk(av_a, 0, hp, n, 256, 384)
                                        normalize_blk(av_b, 64, hp, n, 256, 384)
                                        if endgame and KEG >= 2:
                                            _op_body(4 * n + 2, tail=True)
                                pend = nxt
                            if sim_safe:
                                normalize_blk(av_a, 0, hp, n, 0, 512)
                                normalize_blk(av_b, 64, hp, n, 0, 512)
                                if endgame:
                                    for tt in range(4 * n, 4 * n + 4):
                                        _op_body(tt, tail=(tt >= 4 * n + 2))
                            elif endgame:
                                normalize_blk(av_a, 0, hp, n, 384, 512)
                                normalize_blk(av_b, 64, hp, n, 384, 512)
                                if KEG >= 3:
                                    _op_body(4 * n + 3, tail=True)
                            elif hp == 1 and n == NCHUNK - 1:
                                normalize_blk(av_a, 0, hp, n, 384, 512)
                                normalize_blk(av_b, 64, hp, n, 384, 512)
                            else:
                                def pend_fn(av_a=av_a, av_b=av_b, hp=hp, n=n):
                                    normalize_blk(av_a, 0, hp, n, 384, 512)
                                    normalize_blk(av_b, 64, hp, n, 384, 512)
                                _pend_norm.append(pend_fn)

                    def op_group(tt):
                        def emit():
                            _op_body(tt)
                        return emit

                    def _op_body(tt, tail=False):
                        yt = YT_c[tt // 4]
                        pss = (flexp.tile([P, TC], F32, name="flex"),
                               flexp.tile([P, TC], F32, name="flex"))
                        for a in range(2):
                            for half, ps in ((0, pss[0]), (1, pss[1])):
                                nc.tensor.matmul(
                                    ps,
                                    lhsT=yt[:, a, ts(tt % 4, P)],
                                    rhs=woT_sb[:, a, ts(half, TC)],
                                    start=(a == 0), stop=(a == 1))
                        _op_out(tt, pss, tail=tail)

                    def _op_out(tt, pss, tail=False):
                        # output staged per PAIR of token tiles so each DMA
                        # moves 512KB with 4KB per-partition lines (one
                        # 128KB DMA per half-tile is descriptor- and
                        # line-inefficient and backs up the ring at the
                        # kernel tail).  Tail tiles ship individually so the
                        # last DMA is small and starts as early as possible.
                        if tail:
                            o_sb = obp.tile([P, 2, TC], BF16, name="o_tail")
                            nc.scalar.copy(o_sb[:, 0, :], pss[0])
                            nc.vector.tensor_copy(o_sb[:, 1, :], pss[1])
                            nc.sync.dma_start(
                                out=out[:, tt, :],
                                in_=o_sb[:].rearrange("p a t -> p (a t)"))
                            return
                        if tt % 2 == 0:
                            o_sb = obp.tile([P, 2, 2, TC], BF16, name="o_sb")
                            _ob_cache[tt // 2] = o_sb
                        else:
                            o_sb = _ob_cache.pop(tt // 2)
                        # one cast per engine so an outproj filler body never
                        # head-of-line blocks the next exp on either queue
                        nc.scalar.copy(o_sb[:, tt % 2, 0, :], pss[0])
                        nc.vector.tensor_copy(o_sb[:, tt % 2, 1, :], pss[1])
                        if tt % 2 == 1:
                            nc.sync.dma_start(
                                out=out[:, tt - 1:tt + 1, :],
                                in_=o_sb[:].rearrange("p i a t -> p i (a t)"))

                    proj(0)
                    for n in range(NCHUNK):
                        if n + 1 < NCHUNK:
                            fill_proj.extend(proj_groups(n + 1))
                        attention(n)
                        drain_proj()
                        if n < NCHUNK - 1:
                            fill_op.extend(op_group(tt) for tt in range(4 * n, 4 * n + 4))
                    while fill_op:
                        fill_op.pop(0)()
                    KEG_ = int(__import__('os').environ.get('KEG', '3'))
                    if KEG_ < 3 and not sim_safe:
                        flush_norms()
                        t0 = 4 * (NCHUNK - 1)
                        rest = {0: [0, 1, 2, 3], 1: [2, 3], 2: [3]}[KEG_]
                        for i in rest:
                            _op_body(t0 + i, tail=(i >= 2))
    nc.compile()
    return nc




# ---------------- host-side shard / gather + entry point ----------------

_NC_CACHE = []


def _part(a, p=P):
    """(p*chunks, rest...) -> (p, chunks, rest...) with partition inner."""
    k, rest = a.shape[0], a.shape[1:]
    return np.ascontiguousarray(
        a.reshape(k // p, p, *rest).transpose(1, 0, *range(2, a.ndim + 1)))


def _shard_inputs(x, w_q, w_k, w_v, w_o):
    bf = ml_dtypes.bfloat16
    in_maps = []
    # xT: (P, NCHUNK, KA, TC) — chunk-major so each chunk is one DMA with
    # 8KB contiguous per-partition lines.
    xT_b = [
        np.ascontiguousarray(
            _part(np.ascontiguousarray(np.asarray(x)[b].T).astype(bf))
            .reshape(P, KA, NCHUNK, TC).transpose(0, 2, 1, 3))
        for b in range(B)
    ]
    w_q, w_k, w_v, w_o = (np.asarray(w) for w in (w_q, w_k, w_v, w_o))
    for c in range(8):
        bc, hg = c // 4, c % 4
        r0 = hg * F
        in_maps.append({
            "xT": xT_b[bc],
            "wqT": _part(np.ascontiguousarray(w_q[r0:r0 + F].T).astype(bf)),
            "wkT": _part(np.ascontiguousarray(w_k[r0:r0 + F].T).astype(bf)),
            "wvT": _part(np.ascontiguousarray(w_v[r0:r0 + F].T).astype(bf)),
            "woT": _part(np.ascontiguousarray(w_o[:, r0:r0 + F].T).astype(bf)),
        })
    return in_maps


def _gather(results):
    out = np.zeros((B, T, D), np.float32)
    for c in range(8):
        bc = c // 4
        part = np.asarray(results[c]["out"]).astype(np.float32).reshape(P, NTT, D)
        out[bc] += part.transpose(1, 0, 2).reshape(T, D)
    return out


def kernel(x, w_q, w_k, w_v, w_o):
    from concourse.bass_utils import run_bass_kernel_spmd
    if not _NC_CACHE:
        _NC_CACHE.append(build_nc())
    nc = _NC_CACHE[0]
    in_maps = _shard_inputs(x, w_q, w_k, w_v, w_o)
    res = run_bass_kernel_spmd(nc, in_maps, core_ids=list(range(8)))
    return _gather(res.results)
